# revision 8
# baseline (speedup 1.0000x reference)
"""Distributed Trainium2 kernel for AttHGCNConv:
out = LeakyReLU_0.2( A @ B @ (B.T @ (A.T @ embs)) ),  A=att_adj [N,E], B=inp_adj [E,N].

Never materializes adj = A@B (~1.1 TFLOP); chains 4 thin matmuls of 34 GFLOP
each — memory-bound. 8-way sharded, ALL-GATHER ONLY dataflow (AG on 8 cores
~25us vs ~60us AllReduce):
  S1: t1_c = A[:,e_c].T @ embs        (A col-shard)   -> AG(t1)
  S2: t2[n_c] = B[:,n_c].T @ t1       (B col-shard)   -> AG(t2)
  S3: t3_c = B[e_c,:] @ t2            (B row-shard^T) -> AG(t3)
  S4: out[n_c] = A[n_c,:] @ t3        (A row-shard^T) + fused LeakyReLU,
     f32 rows stored directly; no final collective.

AG/compute pipelining via asymmetric m-part splitting: each stage computes its
output in parts of 3 and 5 m-tiles and all-gathers each part when done, while
the rest computes. 3/8 first-part is the optimum given AG latency ~43us vs
~59us stage compute: the consumer starts when part 1 lands and its part-1
k-work covers the part-2 AG flight. Consumers k-consume in gathered-arrival
order via a host-side permutation of the lhsT k-tile layout. Pass order per
stage is (P1,K1),(P2,K1),(P1,K2),(P2,K2). fp16 operands (PSUM f32), fp16
wires. Bounce buffers are partition-major so every wire-facing DMA is linear.
Gathered-block reads + wire writes ride the scalar-engine DMA queue, apart
from bulk weight prefetch on the sync queue.
"""

import sys

for p in ("/opt/trn_rl_repo", "/root/.axon_site"):
    if p not in sys.path:
        sys.path.insert(0, p)

import numpy as np

import concourse.bass as bass  # noqa: F401
import concourse.mybir as mybir
import concourse.tile as tile
from concourse import bacc
from concourse.bass_utils import run_bass_kernel_spmd

N_CORES = 8
N = 8192  # nodes
E = 8192  # hyperedges
D = 256   # embedding dim
S = E // N_CORES   # 1024 per-core shard
KT = 128           # partition tile
NK = N // KT       # 64 k-tiles
SK = S // KT       # 8 m-tiles
LEAKY = 0.2

MSPLIT = (3, 5)              # m-tiles per output part
MOFF = (0, MSPLIT[0])
KSPLIT = (N_CORES * MSPLIT[0], N_CORES * MSPLIT[1])   # 24, 40 k-tiles
KOFF = (0, KSPLIT[0])

BW_ = 4                      # k-tiles fused per weight DMA
NGH = tuple(k // BW_ for k in KSPLIT)                 # 6, 10 groups per part
EB = 16                      # embs k-tiles per DMA

W16 = mybir.dt.float16       # matmul operand / wire dtype
F32 = mybir.dt.float32
NP16 = np.float16

_CACHED_NC = None


def _build():
    nc = bacc.Bacc("TRN2", target_bir_lowering=False, debug=False,
                   num_devices=N_CORES)

    # two weight tensors per stage (one per m-part, different row widths);
    # rows ordered (hk, g), BW_ k-tiles fused per row
    wg = {}
    for sname in ("a", "b2", "bt", "a2"):
        wg[sname] = [
            nc.dram_tensor(f"{sname}_g{h}", [sum(NGH), KT,
                                             BW_ * MSPLIT[h] * KT], W16,
                           kind="ExternalInput")
            for h in range(2)]
    e_g = nc.dram_tensor("e_g", [NK // EB, KT, EB * D], W16,
                         kind="ExternalInput")
    out = nc.dram_tensor("out", [S, D], F32, kind="ExternalOutput")

    out_v = out.ap().rearrange("(k p) d -> p k d", p=KT)
    rg = [list(range(N_CORES))]

    with tile.TileContext(nc) as tc:
        with (
            tc.tile_pool(name="w", bufs=10) as wpool,
            tc.tile_pool(name="e", bufs=4) as epool,
            tc.tile_pool(name="g", bufs=18) as gpool,
            tc.tile_pool(name="keep", bufs=1) as keep,
            tc.tile_pool(name="ps", bufs=8, space="PSUM") as pspool,
            tc.tile_pool(name="dram", bufs=1, space="DRAM") as dram,
        ):
            # per (stage-boundary, part) bounce buffers; partition-major so
            # rank r's block in the gathered output is rows [128r,128r+128)
            cc_ins = [[dram.tile([KT, MSPLIT[h] * D], W16,
                                 name=f"cci_{i}_{h}", tag=f"cci{i}{h}")
                       for h in range(2)] for i in range(3)]
            cc_outs = [[dram.tile([KT * N_CORES, MSPLIT[h] * D], W16,
                                  addr_space="Shared", name=f"cco_{i}_{h}",
                                  tag=f"cco{i}{h}") for h in range(2)]
                       for i in range(3)]

            # ---- embs preload: 4 x 1MB ----
            es = []
            for ge in range(NK // EB):
                er = epool.tile([KT, EB * D], W16, name="er", tag="e")
                nc.sync.dma_start(er[:], e_g.ap()[ge])
                es.append(er)

            def stage(w2, rhs_of, sink):
                """Pass order (P1,K1),(P2,K1),(P1,K2),(P2,K2). rhs_of(t) maps
                consumption index t (0..63) to an SBUF [128,256] slice.
                sink(hm, m, ps) evicts m-tile MOFF[hm]+m after part hm's
                last pass."""
                ps = [pspool.tile([KT, D], F32, name=f"ps_{m}", tag="ps")
                      for m in range(SK)]
                for hk in range(2):
                    for hm in range(2):
                        mw = MSPLIT[hm] * KT
                        for g in range(NGH[hk]):
                            row = (0 if hk == 0 else NGH[0]) + g
                            w = wpool.tile([KT, BW_ * mw], W16, name="w",
                                           tag=f"w{hm}")
                            nc.sync.dma_start(w[:], w2[hm].ap()[row])
                            for kk in range(BW_):
                                t = KOFF[hk] + g * BW_ + kk
                                rh = rhs_of(t)
                                for m in range(MSPLIT[hm]):
                                    nc.tensor.matmul(
                                        ps[MOFF[hm] + m][:],
                                        w[:, kk * mw + m * KT:
                                          kk * mw + (m + 1) * KT],
                                        rh, start=(t == 0), stop=(t == NK - 1))
                        if hk == 1:  # part hm complete
                            for m in range(MSPLIT[hm]):
                                sink(hm, m, ps[MOFF[hm] + m])

            def gathered_rhs(bidx):
                """rhs from AG boundary bidx: 16 linear block DMAs issued
                eagerly on the scalar queue, gated per part by its AG."""
                blocks = [[], []]
                for hk in range(2):
                    for r in range(N_CORES):
                        b = gpool.tile([KT, MSPLIT[hk] * D], W16, name="gr",
                                       tag=f"g{hk}")
                        nc.scalar.dma_start(
                            b[:], cc_outs[bidx][hk][r * KT:(r + 1) * KT, :])
                        blocks[hk].append(b)

                def rhs(t):
                    hk = 0 if t < KSPLIT[0] else 1
                    r, j = divmod(t - KOFF[hk], MSPLIT[hk])
                    return blocks[hk][r][:, j * D:(j + 1) * D]
                return rhs

            def ag_sink(bidx, t_sb):
                def sink(hm, m, ps):
                    gm = MOFF[hm] + m
                    dst = t_sb[:, gm * D:(gm + 1) * D]
                    if m % 2 == 0:
                        nc.vector.tensor_copy(dst, ps[:])
                    else:
                        nc.scalar.copy(dst, ps[:])
                    if m == MSPLIT[hm] - 1:
                        nc.scalar.dma_start(
                            cc_ins[bidx][hm][:],
                            t_sb[:, MOFF[hm] * D:(MOFF[hm] + MSPLIT[hm]) * D])
                        nc.gpsimd.collective_compute(
                            "AllGather", mybir.AluOpType.bypass,
                            replica_groups=rg,
                            ins=[cc_ins[bidx][hm][:].opt()],
                            outs=[cc_outs[bidx][hm][:].opt()])
                return sink

            # ---- S1: t1 = A[:,e_c].T @ embs ----
            t1 = keep.tile([KT, SK * D], W16, name="t1", tag="t1")
            stage(wg["a"],
                  lambda t: es[t // EB][:, (t % EB) * D:(t % EB + 1) * D],
                  ag_sink(0, t1))

            # ---- S2: t2[n_c] = B[:,n_c].T @ t1_full ----
            t2 = keep.tile([KT, SK * D], W16, name="t2", tag="t2")
            stage(wg["b2"], gathered_rhs(0), ag_sink(1, t2))

            # ---- S3: t3 = B[e_c,:] @ t2_full ----
            t3 = keep.tile([KT, SK * D], W16, name="t3", tag="t3")
            stage(wg["bt"], gathered_rhs(1), ag_sink(2, t3))

            # ---- S4: out[n_c] = A[n_c,:] @ t3_full, LeakyReLU fused ----
            o = keep.tile([KT, SK * D], F32, name="o", tag="o")
            negs = [keep.tile([KT, D], F32, name=f"neg{h}", tag=f"neg{h}")
                    for h in range(2)]

            def leaky_sink(hm, m, ps):
                gm = MOFF[hm] + m
                nc.vector.tensor_scalar_mul(negs[hm][:], ps[:], LEAKY)
                nc.vector.tensor_max(
                    o[:, gm * D:(gm + 1) * D], ps[:], negs[hm][:])
                if m == MSPLIT[hm] - 1:
                    nc.sync.dma_start(
                        out_v[:, MOFF[hm]:MOFF[hm] + MSPLIT[hm], :],
                        o[:, MOFF[hm] * D:(MOFF[hm] + MSPLIT[hm]) * D])

            stage(wg["a2"], gathered_rhs(2), leaky_sink)

    nc.compile()
    return nc


def _relay(w, perm):
    """lhsT [8192, 1024] (k-rows, m-cols) -> two arrays (one per m-part)
    [16, KT, BW_*MSPLIT[h]*KT], k-tiles in consumption order `perm`, rows
    ordered (hk, g)."""
    wt = w.reshape(NK, KT, S)[perm]                    # [64, 128, 1024]
    outs = []
    for h in range(2):
        cols = wt[:, :, MOFF[h] * KT:(MOFF[h] + MSPLIT[h]) * KT]
        parts = []
        for hk in range(2):
            pk = cols[KOFF[hk]:KOFF[hk] + KSPLIT[hk]]  # [K, 128, mw]
            mw = MSPLIT[h] * KT
            parts.append(
                pk.reshape(NGH[hk], BW_, KT, mw).transpose(0, 2, 1, 3)
                .reshape(NGH[hk], KT, BW_ * mw))
        outs.append(np.ascontiguousarray(np.concatenate(parts, axis=0)))
    return outs


# consumption order for gathered rhs: t -> k_global = r*SK + MOFF[hk] + j
_PERM_G = np.array([r * SK + MOFF[hk] + j
                    for hk in range(2) for r in range(N_CORES)
                    for j in range(MSPLIT[hk])])
_PERM_ID = np.arange(NK)


def _fuse_e(eb):
    # [N, D] -> [NK/EB, 128, EB*D]
    return np.ascontiguousarray(
        eb.reshape(NK // EB, EB, KT, D).transpose(0, 2, 1, 3)
    ).reshape(NK // EB, KT, EB * D)


def _shard_inputs(inp_adj, att_adj, embs):
    A = np.asarray(att_adj, dtype=np.float32)   # [N, E]
    B = np.asarray(inp_adj, dtype=np.float32)   # [E, N]
    eb = np.asarray(embs, dtype=np.float32).astype(NP16)   # [N, D]
    e_gh = _fuse_e(eb)
    in_maps = []
    for c in range(N_CORES):
        s = slice(c * S, (c + 1) * S)
        shards = {
            "a": _relay(A[:, s].astype(NP16), _PERM_ID),
            "b2": _relay(B[:, s].astype(NP16), _PERM_G),
            "bt": _relay(np.ascontiguousarray(B[s, :].T).astype(NP16),
                         _PERM_G),
            "a2": _relay(np.ascontiguousarray(A[s, :].T).astype(NP16),
                         _PERM_G),
        }
        m = {"e_g": e_gh}
        for sname, (w0, w1) in shards.items():
            m[f"{sname}_g0"] = w0
            m[f"{sname}_g1"] = w1
        in_maps.append(m)
    return in_maps


def _reset_device():
    """Recover wedged NeuronCores (NRT_EXEC_UNIT_UNRECOVERABLE) via axon."""
    import ctypes

    import jax
    try:
        jax.devices()
        lib = ctypes.CDLL("/opt/axon/libaxon_pjrt.so")
        lib.axon_reset.restype = ctypes.c_int64
        lib.axon_reset()
    except Exception:
        pass


def kernel(inp_adj, att_adj, embs, _trace=False):
    global _CACHED_NC
    if _CACHED_NC is None:
        _CACHED_NC = _build()
    nc = _CACHED_NC
    in_maps = _shard_inputs(inp_adj, att_adj, embs)
    try:
        res = run_bass_kernel_spmd(nc, in_maps,
                                   core_ids=list(range(N_CORES)),
                                   trace=_trace)
    except Exception:
        _reset_device()
        res = run_bass_kernel_spmd(nc, in_maps,
                                   core_ids=list(range(N_CORES)),
                                   trace=_trace)
    # core c owns out rows [c*S, (c+1)*S)
    full = np.empty((N, D), np.float32)
    for c in range(N_CORES):
        full[c * S:(c + 1) * S] = res.results[c]["out"]
    if _trace:
        kernel.last_exec_time_ns = res.exec_time_ns
    return full


# revision 10
# speedup vs baseline: 1.0979x; 1.0979x over previous
"""Distributed Trainium2 kernel for AttHGCNConv:
out = LeakyReLU_0.2( A @ B @ (B.T @ (A.T @ embs)) ),  A=att_adj [N,E], B=inp_adj [E,N].

Never materializes adj = A@B (~1.1 TFLOP); chains 4 thin matmuls of 34 GFLOP
each — memory-bound. 8-way sharded, ALL-GATHER ONLY dataflow (AG on 8 cores
~25us vs ~60us AllReduce):
  S1: t1_c = A[:,e_c].T @ embs        (A col-shard)   -> AG(t1)
  S2: t2[n_c] = B[:,n_c].T @ t1       (B col-shard)   -> AG(t2)
  S3: t3_c = B[e_c,:] @ t2            (B row-shard^T) -> AG(t3)
  S4: out[n_c] = A[n_c,:] @ t3        (A row-shard^T) + fused LeakyReLU,
     f32 rows stored directly; no final collective.

AG/compute pipelining via asymmetric m-part splitting: each stage computes its
output in parts of 3 and 5 m-tiles and all-gathers each part when done, while
the rest computes. 3/8 first-part is the optimum given AG latency ~43us vs
~59us stage compute: the consumer starts when part 1 lands and its part-1
k-work covers the part-2 AG flight. Consumers k-consume in gathered-arrival
order via a host-side permutation of the lhsT k-tile layout. Pass order per
stage is (P1,K1),(P2,K1),(P1,K2),(P2,K2). fp16 operands (PSUM f32), fp16
wires. Bounce buffers are partition-major so every wire-facing DMA is linear.
Gathered-block reads + wire writes ride the scalar-engine DMA queue, apart
from bulk weight prefetch on the sync queue.
"""

import sys

for p in ("/opt/trn_rl_repo", "/root/.axon_site"):
    if p not in sys.path:
        sys.path.insert(0, p)

import numpy as np

import concourse.bass as bass  # noqa: F401
import concourse.mybir as mybir
import concourse.tile as tile
from concourse import bacc
from concourse.bass_utils import run_bass_kernel_spmd

N_CORES = 8
N = 8192  # nodes
E = 8192  # hyperedges
D = 256   # embedding dim
S = E // N_CORES   # 1024 per-core shard
KT = 128           # partition tile
NK = N // KT       # 64 k-tiles
SK = S // KT       # 8 m-tiles
LEAKY = 0.2

MSPLIT = (4, 4)              # m-tiles per output part
MOFF = (0, MSPLIT[0])
KSPLIT = (N_CORES * MSPLIT[0], N_CORES * MSPLIT[1])   # 24, 40 k-tiles
KOFF = (0, KSPLIT[0])

BW_ = 4                      # k-tiles fused per weight DMA
NGH = tuple(k // BW_ for k in KSPLIT)                 # 6, 10 groups per part
EB = 16                      # embs k-tiles per DMA

W16 = mybir.dt.float16       # matmul operand / wire dtype
F32 = mybir.dt.float32
NP16 = np.float16

_CACHED_NC = None


def _build():
    nc = bacc.Bacc("TRN2", target_bir_lowering=False, debug=False,
                   num_devices=N_CORES)

    # two weight tensors per stage (one per m-part, different row widths);
    # rows ordered (hk, g), BW_ k-tiles fused per row
    wg = {}
    for sname in ("a", "b2", "bt", "a2"):
        wg[sname] = [
            nc.dram_tensor(f"{sname}_g{h}", [sum(NGH), KT,
                                             BW_ * MSPLIT[h] * KT], W16,
                           kind="ExternalInput")
            for h in range(2)]
    e_g = nc.dram_tensor("e_g", [NK // EB, KT, EB * D], W16,
                         kind="ExternalInput")
    out = nc.dram_tensor("out", [S, D], F32, kind="ExternalOutput")

    out_v = out.ap().rearrange("(k p) d -> p k d", p=KT)
    rg = [list(range(N_CORES))]

    with tile.TileContext(nc) as tc:
        with (
            tc.tile_pool(name="w", bufs=10) as wpool,
            tc.tile_pool(name="e", bufs=4) as epool,
            tc.tile_pool(name="g", bufs=18) as gpool,
            tc.tile_pool(name="keep", bufs=1) as keep,
            tc.tile_pool(name="ps", bufs=8, space="PSUM") as pspool,
            tc.tile_pool(name="dram", bufs=1, space="DRAM") as dram,
        ):
            # per (stage-boundary, part) bounce buffers; partition-major so
            # rank r's block in the gathered output is rows [128r,128r+128)
            cc_ins = [[dram.tile([KT, MSPLIT[h] * D], W16,
                                 name=f"cci_{i}_{h}", tag=f"cci{i}{h}")
                       for h in range(2)] for i in range(3)]
            cc_outs = [[dram.tile([KT * N_CORES, MSPLIT[h] * D], W16,
                                  addr_space="Shared", name=f"cco_{i}_{h}",
                                  tag=f"cco{i}{h}") for h in range(2)]
                       for i in range(3)]

            # ---- embs preload: 4 x 1MB ----
            es = []
            for ge in range(NK // EB):
                er = epool.tile([KT, EB * D], W16, name="er", tag="e")
                nc.scalar.dma_start(er[:], e_g.ap()[ge])
                es.append(er)

            def stage(w2, rhs_of, sink):
                """Pass order (P1,K1),(P2,K1),(P1,K2),(P2,K2). rhs_of(t) maps
                consumption index t (0..63) to an SBUF [128,256] slice.
                sink(hm, m, ps) evicts m-tile MOFF[hm]+m after part hm's
                last pass."""
                ps = [pspool.tile([KT, D], F32, name=f"ps_{m}", tag="ps")
                      for m in range(SK)]
                for hk in range(2):
                    for hm in range(2):
                        mw = MSPLIT[hm] * KT
                        for g in range(NGH[hk]):
                            row = (0 if hk == 0 else NGH[0]) + g
                            w = wpool.tile([KT, BW_ * mw], W16, name="w",
                                           tag=f"w{hm}")
                            nc.sync.dma_start(w[:], w2[hm].ap()[row])
                            for kk in range(BW_):
                                t = KOFF[hk] + g * BW_ + kk
                                rh = rhs_of(t)
                                for m in range(MSPLIT[hm]):
                                    nc.tensor.matmul(
                                        ps[MOFF[hm] + m][:],
                                        w[:, kk * mw + m * KT:
                                          kk * mw + (m + 1) * KT],
                                        rh, start=(t == 0), stop=(t == NK - 1))
                        if hk == 1:  # part hm complete
                            for m in range(MSPLIT[hm]):
                                sink(hm, m, ps[MOFF[hm] + m])

            def gathered_rhs(bidx):
                """rhs from AG boundary bidx: 16 linear block DMAs issued
                eagerly on the scalar queue, gated per part by its AG."""
                blocks = [[], []]
                for hk in range(2):
                    for r in range(N_CORES):
                        b = gpool.tile([KT, MSPLIT[hk] * D], W16, name="gr",
                                       tag=f"g{hk}")
                        nc.scalar.dma_start(
                            b[:], cc_outs[bidx][hk][r * KT:(r + 1) * KT, :])
                        blocks[hk].append(b)

                def rhs(t):
                    hk = 0 if t < KSPLIT[0] else 1
                    r, j = divmod(t - KOFF[hk], MSPLIT[hk])
                    return blocks[hk][r][:, j * D:(j + 1) * D]
                return rhs

            def ag_sink(bidx, t_sb):
                def sink(hm, m, ps):
                    gm = MOFF[hm] + m
                    dst = t_sb[:, gm * D:(gm + 1) * D]
                    if m % 2 == 0:
                        nc.vector.tensor_copy(dst, ps[:])
                    else:
                        nc.scalar.copy(dst, ps[:])
                    if m == MSPLIT[hm] - 1:
                        nc.scalar.dma_start(
                            cc_ins[bidx][hm][:],
                            t_sb[:, MOFF[hm] * D:(MOFF[hm] + MSPLIT[hm]) * D])
                        nc.gpsimd.collective_compute(
                            "AllGather", mybir.AluOpType.bypass,
                            replica_groups=rg,
                            ins=[cc_ins[bidx][hm][:].opt()],
                            outs=[cc_outs[bidx][hm][:].opt()])
                return sink

            # ---- S1: t1 = A[:,e_c].T @ embs ----
            t1 = keep.tile([KT, SK * D], W16, name="t1", tag="t1")
            stage(wg["a"],
                  lambda t: es[t // EB][:, (t % EB) * D:(t % EB + 1) * D],
                  ag_sink(0, t1))

            # ---- S2: t2[n_c] = B[:,n_c].T @ t1_full ----
            t2 = keep.tile([KT, SK * D], W16, name="t2", tag="t2")
            stage(wg["b2"], gathered_rhs(0), ag_sink(1, t2))

            # ---- S3: t3 = B[e_c,:] @ t2_full ----
            t3 = keep.tile([KT, SK * D], W16, name="t3", tag="t3")
            stage(wg["bt"], gathered_rhs(1), ag_sink(2, t3))

            # ---- S4: out[n_c] = A[n_c,:] @ t3_full, LeakyReLU fused ----
            o = keep.tile([KT, SK * D], F32, name="o", tag="o")
            negs = [keep.tile([KT, D], F32, name=f"neg{h}", tag=f"neg{h}")
                    for h in range(2)]

            def leaky_sink(hm, m, ps):
                gm = MOFF[hm] + m
                nc.vector.tensor_scalar_mul(negs[hm][:], ps[:], LEAKY)
                nc.vector.tensor_max(
                    o[:, gm * D:(gm + 1) * D], ps[:], negs[hm][:])
                if m == MSPLIT[hm] - 1:
                    nc.sync.dma_start(
                        out_v[:, MOFF[hm]:MOFF[hm] + MSPLIT[hm], :],
                        o[:, MOFF[hm] * D:(MOFF[hm] + MSPLIT[hm]) * D])

            stage(wg["a2"], gathered_rhs(2), leaky_sink)

    nc.compile()
    return nc


def _relay(w, perm):
    """lhsT [8192, 1024] (k-rows, m-cols) -> two arrays (one per m-part)
    [16, KT, BW_*MSPLIT[h]*KT], k-tiles in consumption order `perm`, rows
    ordered (hk, g)."""
    wt = w.reshape(NK, KT, S)[perm]                    # [64, 128, 1024]
    outs = []
    for h in range(2):
        cols = wt[:, :, MOFF[h] * KT:(MOFF[h] + MSPLIT[h]) * KT]
        parts = []
        for hk in range(2):
            pk = cols[KOFF[hk]:KOFF[hk] + KSPLIT[hk]]  # [K, 128, mw]
            mw = MSPLIT[h] * KT
            parts.append(
                pk.reshape(NGH[hk], BW_, KT, mw).transpose(0, 2, 1, 3)
                .reshape(NGH[hk], KT, BW_ * mw))
        outs.append(np.ascontiguousarray(np.concatenate(parts, axis=0)))
    return outs


# consumption order for gathered rhs: t -> k_global = r*SK + MOFF[hk] + j
_PERM_G = np.array([r * SK + MOFF[hk] + j
                    for hk in range(2) for r in range(N_CORES)
                    for j in range(MSPLIT[hk])])
_PERM_ID = np.arange(NK)


def _fuse_e(eb):
    # [N, D] -> [NK/EB, 128, EB*D]
    return np.ascontiguousarray(
        eb.reshape(NK // EB, EB, KT, D).transpose(0, 2, 1, 3)
    ).reshape(NK // EB, KT, EB * D)


def _shard_inputs(inp_adj, att_adj, embs):
    A = np.asarray(att_adj, dtype=np.float32)   # [N, E]
    B = np.asarray(inp_adj, dtype=np.float32)   # [E, N]
    eb = np.asarray(embs, dtype=np.float32).astype(NP16)   # [N, D]
    e_gh = _fuse_e(eb)
    in_maps = []
    for c in range(N_CORES):
        s = slice(c * S, (c + 1) * S)
        shards = {
            "a": _relay(A[:, s].astype(NP16), _PERM_ID),
            "b2": _relay(B[:, s].astype(NP16), _PERM_G),
            "bt": _relay(np.ascontiguousarray(B[s, :].T).astype(NP16),
                         _PERM_G),
            "a2": _relay(np.ascontiguousarray(A[s, :].T).astype(NP16),
                         _PERM_G),
        }
        m = {"e_g": e_gh}
        for sname, (w0, w1) in shards.items():
            m[f"{sname}_g0"] = w0
            m[f"{sname}_g1"] = w1
        in_maps.append(m)
    return in_maps


def _reset_device():
    """Recover wedged NeuronCores (NRT_EXEC_UNIT_UNRECOVERABLE) via axon."""
    import ctypes

    import jax
    try:
        jax.devices()
        lib = ctypes.CDLL("/opt/axon/libaxon_pjrt.so")
        lib.axon_reset.restype = ctypes.c_int64
        lib.axon_reset()
    except Exception:
        pass


def kernel(inp_adj, att_adj, embs, _trace=False):
    global _CACHED_NC
    if _CACHED_NC is None:
        _CACHED_NC = _build()
    nc = _CACHED_NC
    in_maps = _shard_inputs(inp_adj, att_adj, embs)
    try:
        res = run_bass_kernel_spmd(nc, in_maps,
                                   core_ids=list(range(N_CORES)),
                                   trace=_trace)
    except Exception:
        _reset_device()
        res = run_bass_kernel_spmd(nc, in_maps,
                                   core_ids=list(range(N_CORES)),
                                   trace=_trace)
    # core c owns out rows [c*S, (c+1)*S)
    full = np.empty((N, D), np.float32)
    for c in range(N_CORES):
        full[c * S:(c + 1) * S] = res.results[c]["out"]
    if _trace:
        kernel.last_exec_time_ns = res.exec_time_ns
    return full


# revision 11
# speedup vs baseline: 1.1199x; 1.0200x over previous
"""Distributed Trainium2 kernel for AttHGCNConv:
out = LeakyReLU_0.2( A @ B @ (B.T @ (A.T @ embs)) ),  A=att_adj [N,E], B=inp_adj [E,N].

Never materializes adj = A@B (~1.1 TFLOP); chains 4 thin matmuls of 34 GFLOP
each — memory-bound. 8-way sharded, ALL-GATHER ONLY dataflow (AG on 8 cores
~25us vs ~60us AllReduce):
  S1: t1_c = A[:,e_c].T @ embs        (A col-shard)   -> AG(t1)
  S2: t2[n_c] = B[:,n_c].T @ t1       (B col-shard)   -> AG(t2)
  S3: t3_c = B[e_c,:] @ t2            (B row-shard^T) -> AG(t3)
  S4: out[n_c] = A[n_c,:] @ t3        (A row-shard^T) + fused LeakyReLU,
     f32 rows stored directly; no final collective.

AG/compute pipelining via asymmetric m-part splitting: each stage computes its
output in parts of 3 and 5 m-tiles and all-gathers each part when done, while
the rest computes. 3/8 first-part is the optimum given AG latency ~43us vs
~59us stage compute: the consumer starts when part 1 lands and its part-1
k-work covers the part-2 AG flight. Consumers k-consume in gathered-arrival
order via a host-side permutation of the lhsT k-tile layout. Pass order per
stage is (P1,K1),(P2,K1),(P1,K2),(P2,K2). fp16 operands (PSUM f32), fp16
wires. Bounce buffers are partition-major so every wire-facing DMA is linear.
Gathered-block reads + wire writes ride the scalar-engine DMA queue, apart
from bulk weight prefetch on the sync queue.
"""

import sys

for p in ("/opt/trn_rl_repo", "/root/.axon_site"):
    if p not in sys.path:
        sys.path.insert(0, p)

import numpy as np

import concourse.bass as bass  # noqa: F401
import concourse.mybir as mybir
import concourse.tile as tile
from concourse import bacc
from concourse.bass_utils import run_bass_kernel_spmd

N_CORES = 8
N = 8192  # nodes
E = 8192  # hyperedges
D = 256   # embedding dim
S = E // N_CORES   # 1024 per-core shard
KT = 128           # partition tile
NK = N // KT       # 64 k-tiles
SK = S // KT       # 8 m-tiles
LEAKY = 0.2

MSPLIT = (4, 4)              # m-tiles per output part
MOFF = (0, MSPLIT[0])
KSPLIT = (N_CORES * MSPLIT[0], N_CORES * MSPLIT[1])   # 24, 40 k-tiles
KOFF = (0, KSPLIT[0])

BW_ = 4                      # k-tiles fused per weight DMA
NGH = tuple(k // BW_ for k in KSPLIT)                 # 6, 10 groups per part
EB = 16                      # embs k-tiles per DMA

W16 = mybir.dt.float16       # matmul operand / wire dtype
F32 = mybir.dt.float32
NP16 = np.float16

_CACHED_NC = None


def _build():
    nc = bacc.Bacc("TRN2", target_bir_lowering=False, debug=False,
                   num_devices=N_CORES)

    # two weight tensors per stage (one per m-part, different row widths);
    # rows ordered (hk, g), BW_ k-tiles fused per row
    wg = {}
    for sname in ("a", "b2", "bt", "a2"):
        wg[sname] = [
            nc.dram_tensor(f"{sname}_g{h}", [sum(NGH), KT,
                                             BW_ * MSPLIT[h] * KT], W16,
                           kind="ExternalInput")
            for h in range(2)]
    e_g = nc.dram_tensor("e_g", [NK // EB, KT, EB * D], W16,
                         kind="ExternalInput")
    out = nc.dram_tensor("out", [S, D], F32, kind="ExternalOutput")

    out_v = out.ap().rearrange("(k p) d -> p k d", p=KT)
    rg = [list(range(N_CORES))]

    with tile.TileContext(nc) as tc:
        with (
            tc.tile_pool(name="w", bufs=10) as wpool,
            tc.tile_pool(name="e", bufs=4) as epool,
            tc.tile_pool(name="g", bufs=18) as gpool,
            tc.tile_pool(name="keep", bufs=1) as keep,
            tc.tile_pool(name="ps", bufs=8, space="PSUM") as pspool,
            tc.tile_pool(name="dram", bufs=1, space="DRAM") as dram,
        ):
            # per (stage-boundary, part) bounce buffers; partition-major so
            # rank r's block in the gathered output is rows [128r,128r+128)
            cc_ins = [[dram.tile([KT, MSPLIT[h] * D], W16,
                                 name=f"cci_{i}_{h}", tag=f"cci{i}{h}")
                       for h in range(2)] for i in range(3)]
            cc_outs = [[dram.tile([KT * N_CORES, MSPLIT[h] * D], W16,
                                  addr_space="Shared", name=f"cco_{i}_{h}",
                                  tag=f"cco{i}{h}") for h in range(2)]
                       for i in range(3)]

            # tiny skew-absorber collective: syncs the cc stream across the
            # 8 cores during S1's compute so the first real AG doesn't pay
            # the cross-core launch-skew wait.
            warm_in = dram.tile([KT, 8], W16, name="warm_in", tag="wi")
            warm_out = dram.tile([KT * N_CORES, 8], W16, addr_space="Shared",
                                 name="warm_out", tag="wo")
            nc.gpsimd.collective_compute(
                "AllGather", mybir.AluOpType.bypass, replica_groups=rg,
                ins=[warm_in[:].opt()], outs=[warm_out[:].opt()])

            # ---- embs preload: 4 x 1MB ----
            es = []
            for ge in range(NK // EB):
                er = epool.tile([KT, EB * D], W16, name="er", tag="e")
                nc.scalar.dma_start(er[:], e_g.ap()[ge])
                es.append(er)

            def stage(w2, rhs_of, sink):
                """Pass order (P1,K1),(P2,K1),(P1,K2),(P2,K2). rhs_of(t) maps
                consumption index t (0..63) to an SBUF [128,256] slice.
                sink(hm, m, ps) evicts m-tile MOFF[hm]+m after part hm's
                last pass."""
                ps = [pspool.tile([KT, D], F32, name=f"ps_{m}", tag="ps")
                      for m in range(SK)]
                for hk in range(2):
                    for hm in range(2):
                        mw = MSPLIT[hm] * KT
                        for g in range(NGH[hk]):
                            row = (0 if hk == 0 else NGH[0]) + g
                            w = wpool.tile([KT, BW_ * mw], W16, name="w",
                                           tag=f"w{hm}")
                            nc.sync.dma_start(w[:], w2[hm].ap()[row])
                            for kk in range(BW_):
                                t = KOFF[hk] + g * BW_ + kk
                                rh = rhs_of(t)
                                for m in range(MSPLIT[hm]):
                                    nc.tensor.matmul(
                                        ps[MOFF[hm] + m][:],
                                        w[:, kk * mw + m * KT:
                                          kk * mw + (m + 1) * KT],
                                        rh, start=(t == 0), stop=(t == NK - 1))
                        if hk == 1:  # part hm complete
                            for m in range(MSPLIT[hm]):
                                sink(hm, m, ps[MOFF[hm] + m])

            def gathered_rhs(bidx):
                """rhs from AG boundary bidx: 16 linear block DMAs issued
                eagerly on the scalar queue, gated per part by its AG."""
                blocks = [[], []]
                for hk in range(2):
                    for r in range(N_CORES):
                        b = gpool.tile([KT, MSPLIT[hk] * D], W16, name="gr",
                                       tag=f"g{hk}")
                        nc.scalar.dma_start(
                            b[:], cc_outs[bidx][hk][r * KT:(r + 1) * KT, :])
                        blocks[hk].append(b)

                def rhs(t):
                    hk = 0 if t < KSPLIT[0] else 1
                    r, j = divmod(t - KOFF[hk], MSPLIT[hk])
                    return blocks[hk][r][:, j * D:(j + 1) * D]
                return rhs

            def ag_sink(bidx, t_sb):
                def sink(hm, m, ps):
                    gm = MOFF[hm] + m
                    dst = t_sb[:, gm * D:(gm + 1) * D]
                    if m % 2 == 0:
                        nc.vector.tensor_copy(dst, ps[:])
                    else:
                        nc.scalar.copy(dst, ps[:])
                    if m == MSPLIT[hm] - 1:
                        nc.scalar.dma_start(
                            cc_ins[bidx][hm][:],
                            t_sb[:, MOFF[hm] * D:(MOFF[hm] + MSPLIT[hm]) * D])
                        nc.gpsimd.collective_compute(
                            "AllGather", mybir.AluOpType.bypass,
                            replica_groups=rg,
                            ins=[cc_ins[bidx][hm][:].opt()],
                            outs=[cc_outs[bidx][hm][:].opt()])
                return sink

            # ---- S1: t1 = A[:,e_c].T @ embs ----
            t1 = keep.tile([KT, SK * D], W16, name="t1", tag="t1")
            stage(wg["a"],
                  lambda t: es[t // EB][:, (t % EB) * D:(t % EB + 1) * D],
                  ag_sink(0, t1))

            # ---- S2: t2[n_c] = B[:,n_c].T @ t1_full ----
            t2 = keep.tile([KT, SK * D], W16, name="t2", tag="t2")
            stage(wg["b2"], gathered_rhs(0), ag_sink(1, t2))

            # ---- S3: t3 = B[e_c,:] @ t2_full ----
            t3 = keep.tile([KT, SK * D], W16, name="t3", tag="t3")
            stage(wg["bt"], gathered_rhs(1), ag_sink(2, t3))

            # ---- S4: out[n_c] = A[n_c,:] @ t3_full, LeakyReLU fused ----
            o = keep.tile([KT, SK * D], F32, name="o", tag="o")
            negs = [keep.tile([KT, D], F32, name=f"neg{h}", tag=f"neg{h}")
                    for h in range(2)]

            def leaky_sink(hm, m, ps):
                gm = MOFF[hm] + m
                nc.vector.tensor_scalar_mul(negs[hm][:], ps[:], LEAKY)
                nc.vector.tensor_max(
                    o[:, gm * D:(gm + 1) * D], ps[:], negs[hm][:])
                if m == MSPLIT[hm] - 1:
                    nc.sync.dma_start(
                        out_v[:, MOFF[hm]:MOFF[hm] + MSPLIT[hm], :],
                        o[:, MOFF[hm] * D:(MOFF[hm] + MSPLIT[hm]) * D])

            stage(wg["a2"], gathered_rhs(2), leaky_sink)

    nc.compile()
    return nc


def _relay(w, perm):
    """lhsT [8192, 1024] (k-rows, m-cols) -> two arrays (one per m-part)
    [16, KT, BW_*MSPLIT[h]*KT], k-tiles in consumption order `perm`, rows
    ordered (hk, g)."""
    wt = w.reshape(NK, KT, S)[perm]                    # [64, 128, 1024]
    outs = []
    for h in range(2):
        cols = wt[:, :, MOFF[h] * KT:(MOFF[h] + MSPLIT[h]) * KT]
        parts = []
        for hk in range(2):
            pk = cols[KOFF[hk]:KOFF[hk] + KSPLIT[hk]]  # [K, 128, mw]
            mw = MSPLIT[h] * KT
            parts.append(
                pk.reshape(NGH[hk], BW_, KT, mw).transpose(0, 2, 1, 3)
                .reshape(NGH[hk], KT, BW_ * mw))
        outs.append(np.ascontiguousarray(np.concatenate(parts, axis=0)))
    return outs


# consumption order for gathered rhs: t -> k_global = r*SK + MOFF[hk] + j
_PERM_G = np.array([r * SK + MOFF[hk] + j
                    for hk in range(2) for r in range(N_CORES)
                    for j in range(MSPLIT[hk])])
_PERM_ID = np.arange(NK)


def _fuse_e(eb):
    # [N, D] -> [NK/EB, 128, EB*D]
    return np.ascontiguousarray(
        eb.reshape(NK // EB, EB, KT, D).transpose(0, 2, 1, 3)
    ).reshape(NK // EB, KT, EB * D)


def _shard_inputs(inp_adj, att_adj, embs):
    A = np.asarray(att_adj, dtype=np.float32)   # [N, E]
    B = np.asarray(inp_adj, dtype=np.float32)   # [E, N]
    eb = np.asarray(embs, dtype=np.float32).astype(NP16)   # [N, D]
    e_gh = _fuse_e(eb)
    in_maps = []
    for c in range(N_CORES):
        s = slice(c * S, (c + 1) * S)
        shards = {
            "a": _relay(A[:, s].astype(NP16), _PERM_ID),
            "b2": _relay(B[:, s].astype(NP16), _PERM_G),
            "bt": _relay(np.ascontiguousarray(B[s, :].T).astype(NP16),
                         _PERM_G),
            "a2": _relay(np.ascontiguousarray(A[s, :].T).astype(NP16),
                         _PERM_G),
        }
        m = {"e_g": e_gh}
        for sname, (w0, w1) in shards.items():
            m[f"{sname}_g0"] = w0
            m[f"{sname}_g1"] = w1
        in_maps.append(m)
    return in_maps


def _reset_device():
    """Recover wedged NeuronCores (NRT_EXEC_UNIT_UNRECOVERABLE) via axon."""
    import ctypes

    import jax
    try:
        jax.devices()
        lib = ctypes.CDLL("/opt/axon/libaxon_pjrt.so")
        lib.axon_reset.restype = ctypes.c_int64
        lib.axon_reset()
    except Exception:
        pass


def kernel(inp_adj, att_adj, embs, _trace=False):
    global _CACHED_NC
    if _CACHED_NC is None:
        _CACHED_NC = _build()
    nc = _CACHED_NC
    in_maps = _shard_inputs(inp_adj, att_adj, embs)
    try:
        res = run_bass_kernel_spmd(nc, in_maps,
                                   core_ids=list(range(N_CORES)),
                                   trace=_trace)
    except Exception:
        _reset_device()
        res = run_bass_kernel_spmd(nc, in_maps,
                                   core_ids=list(range(N_CORES)),
                                   trace=_trace)
    # core c owns out rows [c*S, (c+1)*S)
    full = np.empty((N, D), np.float32)
    for c in range(N_CORES):
        full[c * S:(c + 1) * S] = res.results[c]["out"]
    if _trace:
        kernel.last_exec_time_ns = res.exec_time_ns
    return full
